# revision 5
# baseline (speedup 1.0000x reference)
"""v3: channel-grouped scans over resident 4-chunk super-phases.

Same math as v1 (reversed strided scans, ACT alpha-prescale in place), but
scans are issued channel-major within each phase so consecutive DVE scan
instructions share an identical AP pattern, and 4 chunks are resident at
once (xt/yt bufs=4) for deeper DMA/compute overlap.
"""
import numpy as np

B, T, C = 512, 16384, 3
N_CORES = 8
B_SHARD = B // N_CORES
ROW = T * C
CHUNK_T = 2048
CHUNK_X = CHUNK_T * C
N_CHUNKS = T // CHUNK_T
PHASE = 4                      # chunks per super-phase

_CACHE = {}


def _build(r_vals, alpha_vals, repeat=1):
    from concourse import bacc
    import concourse.tile as tile
    import concourse.mybir as mybir

    nc = bacc.Bacc(trn_type="TRN2", target_bir_lowering=False,
                   num_devices=N_CORES)
    x = nc.declare_dram_parameter("x", [B_SHARD, ROW], mybir.dt.float32,
                                  isOutput=False)
    y = nc.declare_dram_parameter("y", [B_SHARD, ROW], mybir.dt.float32,
                                  isOutput=True)

    with tile.TileContext(nc) as tc:
        with tc.tile_pool(name="cst", bufs=1) as cpool, \
             tc.tile_pool(name="pxt", bufs=PHASE) as pxt, \
             tc.tile_pool(name="pyt", bufs=PHASE) as pyt:
            rts = []
            for c in range(C):
                rt = cpool.tile([B_SHARD, 1], mybir.dt.float32, name=f"r{c}")
                nc.vector.memset(rt[:], float(r_vals[c]))
                rts.append(rt)

            for rep in range(repeat):
                prev_yt = None            # yt of chunk k+1
                for ph in range(N_CHUNKS // PHASE):
                    ks = list(range(N_CHUNKS - 1 - ph * PHASE,
                                    N_CHUNKS - 1 - (ph + 1) * PHASE, -1))
                    xts, yts = {}, {}
                    for k in ks:
                        sl = slice(k * CHUNK_X, (k + 1) * CHUNK_X)
                        xt = pxt.tile([B_SHARD, CHUNK_X], mybir.dt.float32,
                                      name="xt")
                        nc.sync.dma_start(xt[:], x.ap()[:, sl])
                        for c in range(C):
                            nc.scalar.mul(xt[:, c::3], xt[:, c::3],
                                          float(alpha_vals[c]))
                        xts[k] = xt
                        yts[k] = pyt.tile([B_SHARD, CHUNK_X],
                                          mybir.dt.float32, name="yt")
                    for c in range(C):
                        for k in ks:
                            up = yts.get(k + 1, prev_yt)
                            init = 0.0 if up is None else up[:, c:c + 1]
                            nc.vector.tensor_tensor_scan(
                                yts[k][:, c::3][:, ::-1],
                                rts[c][:].to_broadcast([B_SHARD, CHUNK_T]),
                                xts[k][:, c::3][:, ::-1],
                                init,
                                mybir.AluOpType.mult,
                                mybir.AluOpType.add,
                            )
                    for k in ks:
                        sl = slice(k * CHUNK_X, (k + 1) * CHUNK_X)
                        nc.sync.dma_start(y.ap()[:, sl], yts[k][:])
                    prev_yt = yts[ks[-1]]

    nc.compile()
    return nc


def kernel(events, time_decay, alpha):
    import jax.numpy as jnp
    from concourse.bass_utils import run_bass_kernel_spmd

    r_vals = np.asarray(jnp.exp(-1.0 / jnp.asarray(time_decay,
                                                   dtype=jnp.float32)))
    alpha_vals = np.asarray(alpha, dtype=np.float32)
    key = (tuple(r_vals.tolist()), tuple(alpha_vals.tolist()))
    if key not in _CACHE:
        _CACHE[key] = _build(r_vals, alpha_vals)
    nc = _CACHE[key]
    ev = np.ascontiguousarray(events, dtype=np.float32).reshape(B, ROW)
    in_maps = [{"x": ev[i * B_SHARD:(i + 1) * B_SHARD]}
               for i in range(N_CORES)]
    res = run_bass_kernel_spmd(nc, in_maps, list(range(N_CORES)))
    out = np.concatenate([res.results[i]["y"] for i in range(N_CORES)],
                         axis=0)
    return out.reshape(B, T, C)


# revision 6
# speedup vs baseline: 1.3497x; 1.3497x over previous
"""v8 = v3 + (a) stores issued on the ACT HWDGE ring (separate FIFO from
loads on SP), (b) cross-phase carries detached into tiny per-channel tiles
so prev-phase yt buffers release immediately (no slot-allocation stalls)."""
import numpy as np

B, T, C = 512, 16384, 3
N_CORES = 8
B_SHARD = B // N_CORES
ROW = T * C
CHUNK_T = 2048
CHUNK_X = CHUNK_T * C
N_CHUNKS = T // CHUNK_T
PHASE = 4

_CACHE = {}


def _build(r_vals, alpha_vals, repeat=1):
    from concourse import bacc
    import concourse.tile as tile
    import concourse.mybir as mybir

    nc = bacc.Bacc(trn_type="TRN2", target_bir_lowering=False,
                   num_devices=N_CORES)
    x = nc.declare_dram_parameter("x", [B_SHARD, ROW], mybir.dt.float32,
                                  isOutput=False)
    y = nc.declare_dram_parameter("y", [B_SHARD, ROW], mybir.dt.float32,
                                  isOutput=True)

    with tile.TileContext(nc) as tc:
        with tc.tile_pool(name="cst", bufs=1) as cpool, \
             tc.tile_pool(name="pxt", bufs=PHASE) as pxt, \
             tc.tile_pool(name="pyt", bufs=PHASE) as pyt:
            rts, carrs = [], []
            for c in range(C):
                rt = cpool.tile([B_SHARD, 1], mybir.dt.float32, name=f"r{c}")
                nc.vector.memset(rt[:], float(r_vals[c]))
                rts.append(rt)
                ca = cpool.tile([B_SHARD, 1], mybir.dt.float32, name=f"ca{c}")
                carrs.append(ca)

            for rep in range(repeat):
                for ph in range(N_CHUNKS // PHASE):
                    ks = list(range(N_CHUNKS - 1 - ph * PHASE,
                                    N_CHUNKS - 1 - (ph + 1) * PHASE, -1))
                    xts, yts = {}, {}
                    for k in ks:
                        sl = slice(k * CHUNK_X, (k + 1) * CHUNK_X)
                        xt = pxt.tile([B_SHARD, CHUNK_X], mybir.dt.float32,
                                      name="xt")
                        nc.sync.dma_start(xt[:], x.ap()[:, sl])
                        for c in range(C):
                            nc.scalar.mul(xt[:, c::3], xt[:, c::3],
                                          float(alpha_vals[c]))
                        xts[k] = xt
                        yts[k] = pyt.tile([B_SHARD, CHUNK_X],
                                          mybir.dt.float32, name="yt")
                    first = (ph == 0 and rep == repeat - 1) or ph == 0
                    for c in range(C):
                        for k in ks:
                            if k + 1 in yts:
                                init = yts[k + 1][:, c:c + 1]
                            elif ph == 0:
                                init = 0.0
                            else:
                                init = carrs[c][:, 0:1]
                            nc.vector.tensor_tensor_scan(
                                yts[k][:, c::3][:, ::-1],
                                rts[c][:].to_broadcast([B_SHARD, CHUNK_T]),
                                xts[k][:, c::3][:, ::-1],
                                init,
                                mybir.AluOpType.mult,
                                mybir.AluOpType.add,
                            )
                        # detach the carry so yt buffers release promptly
                        if ph < N_CHUNKS // PHASE - 1:
                            nc.scalar.copy(carrs[c][:, 0:1],
                                           yts[ks[-1]][:, c:c + 1])
                    for k in ks:
                        sl = slice(k * CHUNK_X, (k + 1) * CHUNK_X)
                        nc.scalar.dma_start(y.ap()[:, sl], yts[k][:])

    nc.compile()
    return nc


def kernel(events, time_decay, alpha):
    import jax.numpy as jnp
    from concourse.bass_utils import run_bass_kernel_spmd

    r_vals = np.asarray(jnp.exp(-1.0 / jnp.asarray(time_decay,
                                                   dtype=jnp.float32)))
    alpha_vals = np.asarray(alpha, dtype=np.float32)
    key = (tuple(r_vals.tolist()), tuple(alpha_vals.tolist()))
    if key not in _CACHE:
        _CACHE[key] = _build(r_vals, alpha_vals)
    nc = _CACHE[key]
    ev = np.ascontiguousarray(events, dtype=np.float32).reshape(B, ROW)
    in_maps = [{"x": ev[i * B_SHARD:(i + 1) * B_SHARD]}
               for i in range(N_CORES)]
    res = run_bass_kernel_spmd(nc, in_maps, list(range(N_CORES)))
    out = np.concatenate([res.results[i]["y"] for i in range(N_CORES)],
                         axis=0)
    return out.reshape(B, T, C)
